# revision 50
# baseline (speedup 1.0000x reference)
"""CLUB-NCE loss kernel for 8x Trainium2 NeuronCores (Bass/Tile). v2

Math (reference):
  hx = x @ W1x.T, hy = y @ W1y.T            [N, H]
  s[i,j]  = W2 . relu(hy[i] + hx[j] + b1) + b2
  T1[i,j] = softplus(s[i,j]); T0[i] = T1[i,i]
  lower = mean(T0) - (mean_i(logsumexp_j(T1[i,:])) - log N)
  upper = mean(T0) - mean(T1)

Sharding: y rows (i axis) split across 8 cores (64 rows each); x and MLP
params replicated. Each core computes its [64, 512] score block in ONE
psum bank: row i of the block accumulates at psum partition sigma(i),
so the entire epilogue (exp, softplus, row sums, diag extract) runs as a
handful of [128, 512] batched ops instead of 64 per-row ops.

i -> partition map: i = 8u + o (u,o in 0..7); col-group g = o % 4,
phat = o // 4, slot s = 8*phat + u; partition = 32g + s. Each (i, k<3)
matmul uses lhsT = [128, 32] slice of a [128, 48] window tensor that
places w2[k-tile] in output column s; out = PS[32g:32g+32, :] so the
4 col-groups can run concurrently on hardware (tile_position derives
from out.base_partition). The h-tail (h 384..399, 16 rows) is packed
8-i-per-op: partitions (16u + h) with a block-diagonal lhsT.

relu generation is split DVE (tensor_scalar, 4x mode) / ACT (activation
Relu with per-partition bias) to balance engine time.
"""

import numpy as np

N = 512          # number of samples
D = 400          # feature dim
H = 400          # hidden dim
NCORES = 8
NL = N // NCORES  # 64 y-rows per core
KP = 512          # padded contraction dim (d)
KT = 4            # 128-partition d tiles
HT = 3            # full 128-row h tiles (h 0..383); tail h 384..399 packed

ACT_S = (3, 7, 11, 15)  # slots whose (i, k<3) relu ops run on ACT engine


def _sigma(i):
    """i -> psum partition."""
    u, o = i // 8, i % 8
    g, phat = o % 4, o // 4
    return 32 * g + 8 * phat + u


def _build_program(b2val: float, enable_asserts: bool = False):
    import concourse.bacc as bacc
    import concourse.mybir as mybir
    import concourse.tile as tile

    fp16 = mybir.dt.float16
    f32 = mybir.dt.float32
    AF = mybir.ActivationFunctionType
    ALU = mybir.AluOpType

    nc = bacc.Bacc(
        "TRN2",
        target_bir_lowering=False,
        debug=False,
        enable_asserts=enable_asserts,
    )

    # packed inputs (few DMAs: each HWDGE issue costs ~650 ns serialized)
    # xw: per d-tile k: [xt 512 | w1x-m0 128 | w1y-m0 128 | yt 64
    #                   | w2win-k 48 | b1-k 1] = 881 cols
    XWC = 881
    xw_d = nc.dram_tensor("xw_d", [3 * 128, XWC], fp16, kind="ExternalInput")
    xw3_d = nc.dram_tensor("xw3_d", [16, XWC], fp16, kind="ExternalInput")
    # m12: per d-tile k: [w1x-m1 128 | w1y-m1 128 | w1x-m2 128 | w1y-m2 128
    #                    | w1x3 128 | w1yt 16 | w23bd 64 (k=0 only)] = 720
    MC = 720
    m12_d = nc.dram_tensor("m12_d", [4 * 128, MC], fp16, kind="ExternalInput")
    # masks: [maskA 384 | maskB 128] f32 (diag of the score block: cores 0-5
    # have it in the A-region j<384, cores 6-7 in the B-region)
    maskd = nc.dram_tensor("maskd", [128, 512], f32, kind="ExternalInput")

    out_o = nc.dram_tensor("out_o", [128, 3], f32, kind="ExternalOutput")

    hyb3_d = nc.dram_tensor("hyb3_d", [16, NL], f32)  # bounce for h-tail gather

    # natural_log_exp_and_others contains Relu, Exp and Ln: preloading it
    # makes every later activation's table resident (no mid-epilogue switch).
    from concourse.hw_specs import get_activation_tables

    act_sets = list(get_activation_tables(nc.m.arch))
    preload_set = act_sets.index("natural_log_exp_and_others")

    with tile.TileContext(nc) as tc:
        with (
            tc.tile_pool(name="const", bufs=1) as cpool,
            tc.tile_pool(name="work", bufs=56) as wpool,
            tc.tile_pool(name="pbig", bufs=3, space="PSUM") as pbig,
            tc.tile_pool(name="psml", bufs=3, space="PSUM") as psml,
            tc.tile_pool(name="pmain", bufs=1, space="PSUM") as pmain,
            tc.tile_pool(name="pB", bufs=1, space="PSUM") as pB,
        ):
            # preload the table set that covers Relu/Exp/Ln so no activation
            # triggers a mid-kernel table switch
            nc.scalar.add_instruction(
                mybir.InstLoadActFuncSet(
                    name=nc.get_next_instruction_name(),
                    ins=[], outs=[],
                    act_func_set_id=preload_set,
                )
            )

            # ---- packed DMAs, in first-use order ----
            xw = cpool.tile([128, 3 * XWC], fp16, name="xw")
            nc.sync.dma_start(
                out=xw[:, :].rearrange("d (k c) -> d k c", k=3),
                in_=xw_d.ap().rearrange("(k d) c -> d k c", k=3),
            )
            xw3 = cpool.tile([16, XWC], fp16, name="xw3")
            nc.sync.dma_start(out=xw3, in_=xw3_d[:, :])
            m12 = cpool.tile([128, 4 * MC], fp16, name="m12")
            nc.sync.dma_start(
                out=m12[:, :].rearrange("d (k c) -> d k c", k=4),
                in_=m12_d.ap().rearrange("(k d) c -> d k c", k=4),
            )
            maskt = cpool.tile([128, 512], f32, name="maskt")
            nc.sync.dma_start(out=maskt, in_=maskd[:, :])

            def kslice(tile, k, c0, c1, stride):
                if tile is xw and k == 3:
                    return xw3[:, c0:c1]
                if k == 3:
                    # d-tail: only 16 real rows; match xw3's 16-partition APs
                    return tile[0:16, k * stride + c0 : k * stride + c1]
                return tile[:, k * stride + c0 : k * stride + c1]

            xt = [kslice(xw, k, 0, 512, XWC) for k in range(KT)]
            yt = [kslice(xw, k, 768, 832, XWC) for k in range(KT)]
            w1x = [[None] * KT for _ in range(HT)]
            w1y = [[None] * KT for _ in range(HT)]
            for k in range(KT):
                w1x[0][k] = kslice(xw, k, 512, 640, XWC)
                w1y[0][k] = kslice(xw, k, 640, 768, XWC)
                w1x[1][k] = kslice(m12, k, 0, 128, MC)
                w1y[1][k] = kslice(m12, k, 128, 256, MC)
                w1x[2][k] = kslice(m12, k, 256, 384, MC)
                w1y[2][k] = kslice(m12, k, 384, 512, MC)
            w1x3 = [kslice(m12, k, 512, 640, MC) for k in range(KT)]
            w1yt = [kslice(m12, k, 640, 656, MC) for k in range(KT)]
            w2w = [kslice(xw, m, 832, 880, XWC) for m in range(HT)]
            w23 = m12[:, 656:720]
            w2c = [w2w[m][:, 15:16] for m in range(HT)]  # plain w2 columns
            maskA = maskt[:, 0:384]
            maskB = maskt[:, 384:512]
            # tensor_scalar_add needs f32 scalars: up-convert the fp16 b1
            b1f = cpool.tile([128, 4], f32, name="b1f")
            for k in range(HT):
                nc.vector.tensor_copy(
                    out=b1f[:, k : k + 1], in_=kslice(xw, k, 880, 881, XWC)
                )
            nc.vector.tensor_copy(out=b1f[0:16, 3:4], in_=xw3[0:16, 880:881])
            b1t = [b1f[:, k : k + 1] for k in range(KT)]

            # psB holds 4 per-k-tile layers of the B-region scores; every B
            # matmul is a self-contained psum group (start+stop) because a
            # start=True clears has_written for the written partitions across
            # the WHOLE bank, so interleaved open column-groups clobber each
            # other. Layers are summed on DVE in the epilogue.
            psB = pB.tile([128, 512], f32, name="psB")
            b2t = cpool.tile([128, 1], f32, name="b2t")
            nc.vector.memset(b2t, b2val)
            onet = cpool.tile([128, 1], f32, name="onet")
            nc.vector.memset(onet, 1.0)
            n512t = cpool.tile([128, 1], f32, name="n512t")
            nc.vector.memset(n512t, float(N))

            # ---- prologue matmuls: all PE groups up front ----
            hyb = [None] * HT
            hx = [None] * HT
            pyp = [None] * HT
            php = [None] * HT

            def hyb_mm(m):
                pyp[m] = psml.tile([128, NL], f32, name=f"py{m}", tag="pp")  # noqa
                for k in range(KT):
                    nc.tensor.matmul(
                        pyp[m], lhsT=w1y[m][k], rhs=yt[k],
                        start=(k == 0), stop=(k == KT - 1),
                    )

            def hyb_fin(m):
                hyb[m] = cpool.tile([128, NL], f32, name=f"hyb{m}")
                nc.vector.tensor_scalar_add(hyb[m], pyp[m], b1t[m])

            def hx_mm(m):
                php[m] = pbig.tile([128, N], f32, name=f"ph{m}", tag="pp")
                for k in range(KT):
                    nc.tensor.matmul(
                        php[m], lhsT=w1x[m][k], rhs=xt[k],
                        start=(k == 0), stop=(k == KT - 1),
                    )

            def hx_fin(m):
                hx[m] = cpool.tile([128, N], fp16, name=f"hx{m}")
                nc.vector.tensor_copy(out=hx[m], in_=php[m])

            # PE warm-up: the cost model (and HAM on hardware) runs the PE
            # at half rate until ~3us of continuous activity; fill the
            # DMA-wait window with dummy matmuls into the (not yet used)
            # psB bank so the prologue matmuls run at full rate
            warm = cpool.tile([128, 128], fp16, name="warm")
            nc.vector.memset(warm, 0.0)
            for _ in range(28):
                nc.tensor.matmul(
                    psB[0:1, 0:128], lhsT=warm[:, 0:1], rhs=warm,
                    start=True, stop=True, skip_group_check=True,
                )
            # interleave the two m=0 psum groups: consecutive matmuls to the
            # SAME psum tile pay a ~790ns WAW semaphore hop; alternating
            # tiles hides it
            pyp[0] = psml.tile([128, NL], f32, name="py0", tag="pp")
            php[0] = pbig.tile([128, N], f32, name="ph0", tag="pp")
            for k in range(KT):
                nc.tensor.matmul(
                    pyp[0], lhsT=w1y[0][k], rhs=yt[k],
                    start=(k == 0), stop=(k == KT - 1),
                )
                nc.tensor.matmul(
                    php[0], lhsT=w1x[0][k], rhs=xt[k],
                    start=(k == 0), stop=(k == KT - 1),
                )
            hyb_fin(0)
            hx_fin(0)
            hyb3 = cpool.tile([16, NL], f32, name="hyb3")
            hyb3p = cpool.tile([128, 8], f32, name="hyb3p")
            hx3rep = cpool.tile([128, N], fp16, name="hx3rep")
            py3 = [None]
            ph3 = [None]

            # ---- main loop ----
            # A-scheme: j in [0, 384) via w2-slot matmuls into ps rows;
            # B-scheme: j in [384, 512) via r-as-stationary matmuls into psB
            # columns (out free dim 1: PE time is the moving size, so these
            # are nearly free; on HW their LDWEIGHTS hides under A streams).
            NA = 384
            sB0 = cpool.tile([128, 128], f32, name="sB0")
            sB1 = cpool.tile([128, 128], f32, name="sB1")
            ps = pmain.tile([128, NA], f32, name="ps")


            def gen_r(i, k, on_act):
                r = wpool.tile([128, N], fp16, name="r", tag="r")
                if on_act:
                    nc.scalar.activation(
                        out=r, in_=hx[k], func=AF.Relu,
                        bias=hyb[k][:, i : i + 1], scale=1.0,
                    )
                else:
                    nc.vector.tensor_scalar(
                        out=r, in0=hx[k],
                        scalar1=hyb[k][:, i : i + 1], scalar2=0.0,
                        op0=ALU.add, op1=ALU.max,
                    )
                return r

            # late-wave gens pre-issued on the otherwise idle GPSIMD engine
            # (~870 ns/op): they retire several us before their matmuls and
            # directly unload the now-critical DVE/ACT gen stream
            POOLED = ((10, 1), (11, 1), (12, 1), (12, 3), (13, 1), (13, 3),
                      (14, 1), (14, 3), (15, 1), (15, 3))
            pooled = {}
            for k in range(HT):
                for s2, g2 in POOLED:
                    i2 = 8 * (s2 % 8) + g2 + 4 * (s2 // 8)
                    rp = wpool.tile([128, N], fp16, name="rp", tag="r")
                    nc.gpsimd.tensor_scalar(
                        out=rp, in0=hx[k],
                        scalar1=hyb[k][:, i2 : i2 + 1], scalar2=0.0,
                        op0=ALU.add, op1=ALU.max,
                    )
                    pooled[(k, s2, g2)] = rp
                for s in range(16):
                    for g in range(4):
                        u, phat = s % 8, s // 8
                        i = 8 * u + g + 4 * phat
                        sig = 32 * g + s
                        if (k, s, g) in pooled:
                            r = pooled[(k, s, g)]
                        else:
                            r = gen_r(i, k, on_act=(g == 3))
                        nc.tensor.matmul(
                            ps[32 * g : 32 * g + 32, :],
                            lhsT=w2w[k][:, 15 - s : 47 - s],
                            rhs=r[:, 0:NA],
                            start=(k == 0 and s == 0),
                            stop=False,
                            skip_group_check=True,
                            tile_position=(0, 32 * g),
                        )
                        nc.tensor.matmul(
                            psB[:, 128 * k + sig : 128 * k + sig + 1],
                            lhsT=r[:, NA:N],
                            rhs=w2c[k],
                            start=True,
                            stop=True,
                            skip_group_check=True,
                        )
                    if k == 0 and s == 3:
                        # prologue m1/m2/h-tail matmuls, round-robin across
                        # psum tiles to hide WAW semaphore hops; PE consumes
                        # these while DVE/ACT keep generating k=0 waves
                        pyp[1] = psml.tile([128, NL], f32, name="py1", tag="pp")
                        php[1] = pbig.tile([128, N], f32, name="ph1", tag="pp")
                        pyp[2] = psml.tile([128, NL], f32, name="py2", tag="pp")
                        php[2] = pbig.tile([128, N], f32, name="ph2", tag="pp")
                        py3[0] = psml.tile([16, NL], f32, name="py3", tag="pp")
                        ph3[0] = pbig.tile([128, N], f32, name="ph3", tag="pp")
                        for kk in range(KT):
                            nc.tensor.matmul(
                                pyp[1], lhsT=w1y[1][kk], rhs=yt[kk],
                                start=(kk == 0), stop=(kk == KT - 1),
                            )
                            nc.tensor.matmul(
                                php[1], lhsT=w1x[1][kk], rhs=xt[kk],
                                start=(kk == 0), stop=(kk == KT - 1),
                            )
                            nc.tensor.matmul(
                                pyp[2], lhsT=w1y[2][kk], rhs=yt[kk],
                                start=(kk == 0), stop=(kk == KT - 1),
                            )
                            nc.tensor.matmul(
                                php[2], lhsT=w1x[2][kk], rhs=xt[kk],
                                start=(kk == 0), stop=(kk == KT - 1),
                            )
                            nc.tensor.matmul(
                                py3[0], lhsT=w1yt[kk], rhs=yt[kk],
                                start=(kk == 0), stop=(kk == KT - 1),
                            )
                            nc.tensor.matmul(
                                ph3[0], lhsT=w1x3[kk], rhs=xt[kk],
                                start=(kk == 0), stop=(kk == KT - 1),
                            )
                    if k == 0 and s == 6:
                        # deferred DVE finishes (prologue matmuls retired by
                        # now; placing them here keeps the DVE FIFO stall-free)
                        hyb_fin(1)
                        hx_fin(1)
                        hyb_fin(2)
                        hx_fin(2)
                        nc.vector.tensor_scalar_add(hyb3, py3[0], b1f[0:16, 3:4])
                        nc.vector.tensor_copy(out=hx3rep, in_=ph3[0])
                        # hyb3p[16u + h, o] = hyb3[h, 8u + o] via DRAM bounce
                        nc.sync.dma_start(out=hyb3_d[:, :], in_=hyb3)
                        for u in range(8):
                            nc.sync.dma_start(
                                out=hyb3p[16 * u : 16 * u + 16, :],
                                in_=hyb3_d[:, 8 * u : 8 * u + 8],
                            )
                    if k == 1 and s == 8:
                        # k=0 B-layer retired: start the layer sum off-tail
                        nc.vector.tensor_copy(out=sB0, in_=psB[:, 0:128])
                    if k == 2 and s == 8:
                        nc.vector.tensor_tensor(
                            out=sB1, in0=sB0, in1=psB[:, 128:256], op=ALU.add
                        )
            # h-tail: one packed relu per 8 rows; block-diag A matmul and
            # an 8-column B matmul
            for phat in range(2):
                for g in range(4):
                    o = g + 4 * phat
                    r3 = wpool.tile([128, N], fp16, name="r3", tag="r")
                    if g % 2 == 0:
                        nc.scalar.activation(
                            out=r3, in_=hx3rep, func=AF.Relu,
                            bias=hyb3p[:, o : o + 1], scale=1.0,
                        )
                    else:
                        nc.vector.tensor_scalar(
                            out=r3, in0=hx3rep,
                            scalar1=hyb3p[:, o : o + 1], scalar2=0.0,
                            op0=ALU.add, op1=ALU.max,
                        )
                    nc.tensor.matmul(
                        ps[32 * g : 32 * g + 32, :],
                        lhsT=w23[:, 32 * phat : 32 * phat + 32],
                        rhs=r3[:, 0:NA],
                        start=False,
                        stop=(phat == 1),
                        skip_group_check=True,
                        tile_position=(0, 32 * g),
                    )
                    nc.tensor.matmul(
                        psB[:, 384 + 32 * g + 8 * phat : 384 + 32 * g + 8 * phat + 8],
                        lhsT=r3[:, NA:N],
                        rhs=w23[:, 0:8],
                        start=True,
                        stop=True,
                        skip_group_check=True,
                    )

            # ---- epilogue: batched over all 64 rows at once ----
            out3 = cpool.tile([128, 3], f32, name="out3")
            ecols = cpool.tile([128, NA], f32, name="ecols")
            rsum_e = cpool.tile([128, 1], f32, name="rsum_e")
            rsA = cpool.tile([128, 1], f32, name="rsA")
            EB = cpool.tile([128, 128], fp16, name="EB")
            T1B = cpool.tile([128, 128], fp16, name="T1B")
            mEB = cpool.tile([128, 128], fp16, name="mEB")
            maskB16 = cpool.tile([128, 128], fp16, name="maskB16")
            nc.vector.tensor_copy(out=maskB16, in_=maskB)
            ones16 = cpool.tile([128, 1], fp16, name="ones16")
            nc.vector.memset(ones16, 1.0)
            # E = exp(s + b2) for both regions; A row sums via accum
            nc.scalar.activation(
                out=ecols, in_=ps, func=AF.Exp, bias=b2t, scale=1.0,
                accum_out=rsum_e,
            )
            # sum the 4 B layers (copy + 3 adds; only one PSUM read port)
            nc.vector.tensor_copy(out=sB0, in_=psB[:, 0:128])
            nc.vector.tensor_tensor(out=sB1, in0=sB0, in1=psB[:, 128:256], op=ALU.add)
            sB2 = cpool.tile([128, 128], f32, name="sB2")
            nc.vector.tensor_tensor(out=sB2, in0=sB1, in1=psB[:, 256:384], op=ALU.add)
            sB3 = cpool.tile([128, 128], f32, name="sB3")
            nc.vector.tensor_tensor(out=sB3, in0=sB2, in1=psB[:, 384:512], op=ALU.add)

            # diag extract (DVE, parallel with the Ln ops below)
            junk = cpool.tile([128, NA], f32, name="junk")
            nc.vector.tensor_tensor(out=junk, in0=ecols, in1=maskA, op=ALU.mult)
            ediag = cpool.tile([128, 1], f32, name="ediag")
            nc.vector.reduce_sum(out=ediag, in_=junk, axis=mybir.AxisListType.X)
            nc.scalar.activation(out=EB, in_=sB3, func=AF.Exp, bias=b2t, scale=1.0)
            nc.vector.tensor_tensor(out=mEB, in0=EB, in1=maskB16, op=ALU.mult)
            # T1 = ln(1 + E); A row sums via accum
            t1 = cpool.tile([128, NA], f32, name="t1")
            nc.scalar.activation(
                out=t1, in_=ecols, func=AF.Ln, bias=onet, scale=1.0,
                accum_out=rsA,
            )
            nc.scalar.activation(out=T1B, in_=EB, func=AF.Ln, bias=onet, scale=1.0)
            # B-region row sums: transposed layout, so reduce over j
            # (partitions) with ones-matmuls; columns are already sigma(i)
            rsB3 = psml.tile([128, 3], f32, name="rsB3", tag="pp")
            nc.tensor.matmul(rsB3[:, 0:1], lhsT=EB, rhs=ones16,
                             start=True, stop=True, skip_group_check=True)
            nc.tensor.matmul(rsB3[:, 1:2], lhsT=T1B, rhs=ones16,
                             start=True, stop=True, skip_group_check=True)
            nc.tensor.matmul(rsB3[:, 2:3], lhsT=mEB, rhs=ones16,
                             start=True, stop=True, skip_group_check=True)
            # combine A + B partials
            rsum_t = cpool.tile([128, 1], f32, name="rsum_t")
            nc.vector.tensor_tensor(out=rsum_t, in0=rsum_e, in1=rsB3[:, 0:1], op=ALU.add)
            nc.vector.tensor_tensor(out=out3[:, 1:2], in0=rsA, in1=rsB3[:, 1:2], op=ALU.add)
            ediag_t = cpool.tile([128, 1], f32, name="ediag_t")
            nc.vector.tensor_tensor(out=ediag_t, in0=ediag, in1=rsB3[:, 2:3], op=ALU.add)
            # row logsumexp = ln(512 + sum_j e^s); T0 = ln(1 + E_diag)
            nc.scalar.activation(
                out=out3[:, 0:1], in_=rsum_t, func=AF.Ln, bias=n512t, scale=1.0
            )
            nc.scalar.activation(
                out=out3[:, 2:3], in_=ediag_t, func=AF.Ln, bias=onet, scale=1.0
            )
            nc.sync.dma_start(out=out_o[:, :], in_=out3)

    nc.compile()
    return nc


def _make_in_maps(x, y, W1, b1, W2):
    f16 = np.float16
    XWC, MC = 881, 720
    xTp = np.zeros((KP, N), f16)          # [d, j] = x.T, d-padded
    xTp[:D, :] = x.T.astype(f16)
    w1xTp = np.zeros((KP, H), f16)        # [d, h] = W1x.T
    w1xTp[:D, :] = W1[:, :D].T.astype(f16)
    w1yTp = np.zeros((KP, H), f16)
    w1yTp[:D, :] = W1[:, D:].T.astype(f16)
    b1p = np.zeros((KP,), np.float32)
    b1p[:H] = b1

    # m12: [w1x-m1 | w1y-m1 | w1x-m2 | w1y-m2 | w1x3rep | w1yt | w23] = 720
    m12 = np.zeros((KP, MC), f16)
    m12[:, 0:128] = w1xTp[:, 128:256]
    m12[:, 128:256] = w1yTp[:, 128:256]
    m12[:, 256:384] = w1xTp[:, 256:384]
    m12[:, 384:512] = w1yTp[:, 256:384]
    m12[:, 512:640] = np.tile(w1xTp[:, 384:400], (1, 8))
    m12[:, 640:656] = w1yTp[:, 384:400]
    # w23bd in the k=0 chunk only (rows 0..128 = d-tile 0)
    for phat in range(2):
        for u in range(8):
            for h in range(16):
                m12[16 * u + h, 656 + 32 * phat + 8 * phat + u] = np.float16(
                    W2[0, 384 + h]
                )

    in_maps = []
    for c in range(NCORES):
        yTp = np.zeros((KP, NL), f16)
        yTp[:D, :] = y[c * NL : (c + 1) * NL, :].T.astype(f16)
        # xw: [xt 512 | w1x-m0 128 | w1y-m0 128 | yt 64 | w2win-k 48 | b1 1]
        xw = np.zeros((KP, XWC), f16)
        xw[:, 0:512] = xTp
        xw[:, 512:640] = w1xTp[:, 0:128]
        xw[:, 640:768] = w1yTp[:, 0:128]
        xw[:, 768:832] = yTp
        for k in range(HT):
            xw[k * 128 : (k + 1) * 128, 832 + 15] = W2[0, k * 128 : (k + 1) * 128].astype(f16)
        for k in range(KT):
            xw[k * 128 : (k + 1) * 128, 880] = b1p[k * 128 : (k + 1) * 128].astype(f16)
        # [maskA 384 | maskB 128]: diag element of row i is at global column
        # j = c*64 + i; A-region if j < 384 else B-region (transposed: row
        # j-384, column sigma(i))
        maskp = np.zeros((128, 512), np.float32)
        for i in range(NL):
            j = c * NL + i
            if j < 384:
                maskp[_sigma(i), j] = 1.0
            else:
                maskp[j - 384, 384 + _sigma(i)] = 1.0
        in_maps.append(
            {"xw_d": xw[0:384].copy(), "xw3_d": xw[384:400].copy(),
             "m12_d": m12, "maskd": maskp}
        )
    return in_maps


def _combine(results):
    perm = np.array([_sigma(i) for i in range(NL)])
    lse_all = np.concatenate(
        [r["out_o"][perm, 0].astype(np.float64) for r in results]
    )
    rs_all = np.concatenate(
        [r["out_o"][perm, 1].astype(np.float64) for r in results]
    )
    t0_all = np.concatenate(
        [r["out_o"][perm, 2].astype(np.float64) for r in results]
    )
    t0_mean = t0_all.mean()
    lower = t0_mean - (lse_all.mean() - np.log(np.float64(N)))
    upper = t0_mean - rs_all.mean() / N
    return np.float32(lower), np.float32(upper)


def kernel(x_samples, y_samples, W1, b1, W2, b2, _trace=False):
    from concourse.bass_utils import run_bass_kernel_spmd

    nc = _build_program(float(np.float32(b2[0])))
    in_maps = _make_in_maps(
        np.asarray(x_samples, np.float32),
        np.asarray(y_samples, np.float32),
        np.asarray(W1, np.float32),
        np.asarray(b1, np.float32),
        np.asarray(W2, np.float32),
    )
    res = run_bass_kernel_spmd(
        nc, in_maps, core_ids=list(range(NCORES)), trace=_trace
    )
    out = _combine(res.results)
    if _trace:
        return out, res
    return out
